# revision 1
# baseline (speedup 1.0000x reference)
"""Trainium2 Bass kernel for nn_PredictionModel (CPC-style prediction scores).

Computation (B=4, L=512, D=512, C=256, K=12, LW=500):
  c_proj[b,l,k,d] = sum_c Wk[k,d,c] * c[b,l,c]          (l < LW)
  zw[b,l,k,d]     = z[b, l+1+k, d]
  pos[b,l,k]      = <c_proj[b,l,k], zw[b,l,k]>
  neg_g[b,n,l,k]  = <c_proj[b,l,k], zw[perm_B[n], perm_L[l], k]>
  neg_len[b,n,l,k]= <c_proj[b,l,k], zw[b, perms_len[n,l], k]>
  out = concat([pos[:,None], neg_g, neg_len], axis=1)   # (B, 9, LW, K)

Sharding: 8 cores = 4 batches x 2 l-ranges ([0,256) and [244,500), padded to
256 rows each; host takes l<250 from half 0 and l>=250 from half 1).
Uniform program; all per-core differences arrive via input tensors.
"""

import numpy as np
import ml_dtypes

import concourse.mybir as mybir
from concourse import bacc
from concourse.tile import TileContext
from concourse import bass_utils

B, L, D, C, K = 4, 512, 512, 256, 12
LW = L - K          # 500
LH = 256            # padded per-core l count
L0S = [0, 244]      # absolute start of each half
NM = 2 * B + 1      # 9 score rows per (l, k)
F32 = mybir.dt.float32
BF16 = mybir.dt.bfloat16
BF16_NP = ml_dtypes.bfloat16


_NC = None

# tuning knobs (sim-ablation support)
CFG = {
    "mul_dve_every": 1,   # u % N == N-1 -> DVE mul, else gpsimd (1 = all DVE)
    "red_act_every": 2,   # m % N == N-1 -> ACT reduce, else DVE
    "kg": 6,              # k's per c_proj group
    "do_mul": True,
    "do_reduce": True,
    "do_dots_dma": True,
    "halving_add": True,
    "gp_units": (),
    "halving_act": True,
    "halving2": True,
    "act_ms": None,
}


def _build_program(cfg=None):
    """One NeuronCore program, identical across the 8 cores."""
    global _NC
    if cfg is not None:
        pass
    elif _NC is not None:
        return _NC
    cfg = {**CFG, **(cfg or {})}
    nc = bacc.Bacc()
    # [c-part 128, c-chunk 2, l 256] stationary operand (c[b,half].T)
    ct_d = nc.dram_tensor("ct", [128, 2, LH], F32, kind="ExternalInput")
    # [c-part 128, k 12, c-chunk 2, d 512] moving operand (Wk[k].T)
    wkt_d = nc.dram_tensor("wkt", [128, K, 2, D], F32, kind="ExternalInput")
    # pre-gathered z windows, bf16: [m 9, blk 2, l-part 128, k 12, d 512]
    zw_d = nc.dram_tensor("zw", [NM, 2, 128, K, D], BF16, kind="ExternalInput")
    out_d = nc.dram_tensor("out", [2, 2, 128, NM * K], F32, kind="ExternalOutput")

    with TileContext(nc) as tc:
        with (
            tc.tile_pool(name="const", bufs=1) as const_pool,
            tc.tile_pool(name="cproj", bufs=1) as cproj_pool,
            tc.tile_pool(name="psum", bufs=8, space="PSUM") as psum_pool,
            tc.tile_pool(name="zw", bufs=cfg.get("zw_bufs", 6)) as zw_pool,
            tc.tile_pool(name="prod", bufs=5) as prod_pool,
            tc.tile_pool(name="half", bufs=6) as half_pool,
            tc.tile_pool(name="junk", bufs=4) as junk_pool,
            tc.tile_pool(name="scores", bufs=1) as scores_pool,
        ):
            ct_sb = const_pool.tile([128, 2, LH], F32, tag="ct", name="ct_sb")
            nc.sync.dma_start(out=ct_sb[:], in_=ct_d[:])
            wkt_sb = const_pool.tile([128, K, 2, D], F32, tag="wkt", name="wkt_sb")
            for kg_ in range(3):
                nc.sync.dma_start(
                    out=wkt_sb[:, kg_ * 4 : (kg_ + 1) * 4],
                    in_=wkt_d[:, kg_ * 4 : (kg_ + 1) * 4],
                )

            # c_proj[(blk, kg)]: [l 128, KG k's, d 512] fp32 matmuls -> bf16
            KG = cfg.get("kg", 4)  # k's per group
            NG = K // KG
            cproj = {}
            for kg in range(NG):
                for blk in range(2):
                    cproj[(blk, kg)] = cproj_pool.tile(
                        [128, KG, D], BF16, tag=f"cp{blk}_{kg}", name=f"cp{blk}_{kg}"
                    )
            for kg in range(NG):
                for blk in range(2):
                    for ki in range(KG):
                        k = kg * KG + ki
                        ps = psum_pool.tile(
                            [128, D], F32, name=f"ps{k}_{blk}", tag="ps"
                        )
                        for ci in range(2):
                            nc.tensor.matmul(
                                ps[:],
                                ct_sb[:, ci, blk * 128 : (blk + 1) * 128],
                                wkt_sb[:, k, ci, :],
                                start=(ci == 0),
                                stop=(ci == 1),
                            )
                        # psum->sbuf cast copies on ACT (keeps DVE free)
                        nc.scalar.copy(cproj[(blk, kg)][:, ki, :], ps[:])

            scores = {}
            for par in range(2):
                for blk in range(2):
                    scores[(par, blk)] = scores_pool.tile(
                        [128, NM * K], F32, tag=f"s{par}_{blk}", name=f"s{par}_{blk}"
                    )

            # one unit = (m, blk): mul [128, K*D] + (maybe halving add) + reduce
            units = [(m, blk) for m in range(NM) for blk in range(2)]
            mde, rae = cfg["mul_dve_every"], cfg["red_act_every"]
            zts = {}
            for u, (m, blk) in enumerate(units):
                if cfg.get("merge_zw") :
                    if blk == 0:
                        ztm = zw_pool.tile(
                            [128, 2, K, D], BF16, tag="zw", name=f"ztm{m}"
                        )
                        if cfg["do_dots_dma"]:
                            dma_eng = nc.sync if m % 2 == 0 else nc.scalar
                            dma_eng.dma_start(
                                out=ztm[:],
                                in_=zw_d[m].rearrange("b p k d -> p b k d"),
                            )
                        zts[m] = ztm
                    zt = zts[m][:, blk]
                else:
                    zt = zw_pool.tile([128, K, D], BF16, tag="zw", name=f"zt{u}")
                    if cfg["do_dots_dma"]:
                        dma_eng = nc.sync if u % 2 == 0 else nc.scalar
                        dma_eng.dma_start(out=zt[:], in_=zw_d[m, blk])
                if not cfg["do_mul"]:
                    continue
                mul_eng = nc.gpsimd if u in cfg["gp_units"] else (nc.vector if (mde and u % mde == mde - 1) else nc.gpsimd)
                on_act = (m in cfg["act_ms"]) if cfg.get("act_ms") is not None else (rae and m % rae == rae - 1)
                for kg in range(NG):
                    prod = prod_pool.tile(
                        [128, KG, D], BF16, tag="prod", name=f"pr{u}_{kg}"
                    )
                    mul_eng.tensor_tensor(
                        out=prod[:],
                        in0=cproj[(blk, kg)][:],
                        in1=zt[:, kg * KG : (kg + 1) * KG, :],
                        op=mybir.AluOpType.mult,
                    )
                    if not cfg["do_reduce"]:
                        continue
                    if cfg["halving_add"] and (cfg["halving_act"] or not on_act):
                        a1 = half_pool.tile(
                            [128, KG, D // 2], BF16, tag="half", name=f"a1{u}_{kg}"
                        )
                        nc.vector.tensor_tensor(
                            out=a1[:], in0=prod[:, :, 0 : D // 2],
                            in1=prod[:, :, D // 2 : D], op=mybir.AluOpType.add,
                        )
                        red_in, rw = a1, D // 2
                        if cfg.get("halving2") and not on_act:
                            a2 = half_pool.tile(
                                [128, KG, D // 4], BF16, tag="half2",
                                name=f"a2{u}_{kg}",
                            )
                            nc.vector.tensor_tensor(
                                out=a2[:], in0=a1[:, :, 0 : D // 4],
                                in1=a1[:, :, D // 4 : D // 2],
                                op=mybir.AluOpType.add,
                            )
                            red_in, rw = a2, D // 4
                    else:
                        red_in, rw = prod, D
                    c0 = m * K + kg * KG
                    if not on_act:
                        nc.vector.tensor_reduce(
                            out=scores[(m % 2, blk)][:, c0 : c0 + KG],
                            in_=red_in[:],
                            axis=mybir.AxisListType.X,
                            op=mybir.AluOpType.add,
                        )
                    else:
                        for ki in range(KG):
                            junk = junk_pool.tile(
                                [128, rw], BF16, tag="junk", name=f"j{u}_{kg}_{ki}"
                            )
                            nc.scalar.activation(
                                out=junk[:],
                                in_=red_in[:, ki, :],
                                func=mybir.ActivationFunctionType.Copy,
                                accum_out=scores[(m % 2, blk)][
                                    :, c0 + ki : c0 + ki + 1
                                ],
                            )

            for par in range(2):
                for blk in range(2):
                    nc.sync.dma_start(
                        out=out_d[par, blk], in_=scores[(par, blk)][:]
                    )

    nc.compile()
    if cfg == CFG:
        _NC = nc
    return nc


def _make_inputs(c, z, Wk, perms_len, perm_L, perm_B):
    """Host-side sharding: per-core input dicts."""
    z_bf = z.astype(BF16_NP)
    wkt = np.ascontiguousarray(
        Wk.transpose(0, 2, 1).reshape(K, 2, 128, D).transpose(2, 0, 1, 3)
    )  # [128, K, 2, D]
    karr = np.arange(K, dtype=np.int64)[None, :]
    in_maps = []
    for b in range(B):
        for h in range(2):
            L0 = L0S[h]
            l_abs = np.arange(L0, L0 + LH, dtype=np.int64)
            ct = np.ascontiguousarray(
                c[b, L0 : L0 + LH, :].T.reshape(2, 128, LH).transpose(1, 0, 2)
            )  # [128, 2, LH]
            zw = np.empty((NM, 2, 128, K, D), dtype=BF16_NP)
            for m in range(NM):
                if m == 0:
                    sb, sl = b, l_abs
                elif m <= B:
                    sb, sl = int(perm_B[m - 1]), perm_L[l_abs].astype(np.int64)
                else:
                    sb, sl = b, perms_len[m - 1 - B, l_abs].astype(np.int64)
                rows = sl[:, None] + 1 + karr  # (LH, K)
                zw[m] = z_bf[sb, rows].reshape(2, 128, K, D)
            in_maps.append({"ct": ct, "wkt": wkt, "zw": zw})
    return in_maps


def kernel(c, z, Wk, perms_len, perm_L, perm_B, _trace=False, _result_holder=None):
    c = np.asarray(c, np.float32)
    z = np.asarray(z, np.float32)
    Wk = np.asarray(Wk, np.float32)
    perms_len = np.asarray(perms_len)
    perm_L = np.asarray(perm_L)
    perm_B = np.asarray(perm_B)

    nc = _build_program()
    in_maps = _make_inputs(c, z, Wk, perms_len, perm_L, perm_B)
    res = bass_utils.run_bass_kernel_spmd(
        nc, in_maps, core_ids=list(range(2 * B)), trace=_trace
    )
    if _result_holder is not None:
        _result_holder.append(res)

    out = np.empty((B, NM, LW, K), np.float32)
    for b in range(B):
        for h in range(2):
            r = res.results[2 * b + h]["out"].reshape(2, LH, NM, K)
            merged = np.empty((LH, NM, K), np.float32)
            for m in range(NM):
                merged[:, m] = r[m % 2, :, m]
            s = merged.transpose(1, 0, 2)
            if h == 0:
                out[b, :, :250, :] = s[:, :250, :]
            else:
                out[b, :, 250:, :] = s[:, 250 - L0S[1] :, :]
    return out



# revision 9
# speedup vs baseline: 1.9872x; 1.9872x over previous
"""Trainium2 Bass kernel for nn_PredictionModel (CPC-style prediction scores).

Reference computation (B=4, L=512, D=512, C=256, K=12, LW=500):
  cp[b,l,k,:]    = c[b,l,:] @ Wk[k].T            (row of R^D)
  zw[b,l,k,:]    = z[b, l+1+k, :]
  pos[b,l,k]     = <cp[b,l,k], zw[b,l,k]>
  neg_g[b,n,l,k] = <cp[b,l,k], zw[perm_B[n], perm_L[l], k]>
  neg_len[b,n,l,k]=<cp[b,l,k], zw[b, perms_len[n,l], k]>
  out = concat([pos[:,None], neg_g, neg_len], axis=1)   # (B, 9, LW, K)

Key algebraic move (C-space dots): <c[l] @ Wk[k].T, z[r]> = <c[l], z[r] @ Wk[k]>.
Define zp[r,k,:] = z[r,:] @ Wk[k] in R^C and the k-shifted table
zps[q,k,:] = zp[q+k,k,:]; then every score is
  score[...,l,k] = <c[b_out, l, :], zps_{b_src}[perm(l)+1, k, :]>
so one 6KB-contiguous row of zps serves all 12 k of a score row, and the
dot length is C=256 instead of D=512.

Per-core plan (8 cores = 4 source-batches x 2 q-halves):
  - PE computes zps_{b_src} for its 2 q-blocks directly in shifted layout
    (lhsT = z^T columns offset by k; 96 matmuls, bf16).
  - One dma_gather pulls the c rows (512B each) for all 9 units straight
    from HBM in q-natural order (indices baked host-side from the perms).
  - Dots: bf16 mul (c broadcast over k) + halving adds + reduce, split
    across DVE / ACT(accum) / Pool(fused stt) by a tunable mode string.
  - Scores are emitted q-indexed; the host un-permutes (pure indexing).
"""

import numpy as np
import ml_dtypes

import concourse.mybir as mybir
from concourse import bacc
from concourse.tile import TileContext
from concourse import bass_utils

B, L, D, C, K = 4, 512, 512, 256, 12
LW = L - K            # 500
NM = 2 * B + 1        # 9 output channels
NU = 9                # units per source batch
NQB = 2               # q-blocks per core
NI = NU * NQB * 128   # gather slots per core = 2304
COLS = NI // 16       # idx columns = 144
ZT_PAD = 528          # 512 + 16 (k-shift slack)
F32 = mybir.dt.float32
BF16 = mybir.dt.bfloat16
I16 = mybir.dt.int16
BF16_NP = ml_dtypes.bfloat16

_NC = None

# engine mode per (unit, qj) flat index i = u*2+qj:
#   'd' = DVE mul+halve+halve+reduce
#   'a' = DVE mul+halve+halve, ACT per-k accum tail
#   'p' = Pool fused scalar_tensor_tensor per k
CFG = {
    "modes": "hadhadhadhadhadhad",
    "prod_bufs": 4,
    "junk_bufs": 4,
}


def _build_program(cfg=None):
    """One NeuronCore program, identical across the 8 cores."""
    global _NC
    if cfg is None and _NC is not None:
        return _NC
    cfg = {**CFG, **(cfg or {})}
    modes = cfg["modes"]
    assert len(modes) == NU * NQB

    nc = bacc.Bacc()
    # z[b_src]^T padded: [128 d-part, 4 d-chunk, 528 r]
    zt_d = nc.dram_tensor("zt", [128, 4, ZT_PAD], BF16, kind="ExternalInput")
    # Wk transposed: [128 d-part, 4 d-chunk, K, C]
    wk_d = nc.dram_tensor("wk", [128, 4, K, C], BF16, kind="ExternalInput")
    # all batches' c rows: [B*L, C]
    call_d = nc.dram_tensor("call", [B * L, C], BF16, kind="ExternalInput")
    # gather index table (wrap-16 slots, replicated to all 8 Q7 groups)
    idx_d = nc.dram_tensor("idx", [128, COLS], I16, kind="ExternalInput")
    out_d = nc.dram_tensor("out", [128, NU, NQB, K], F32, kind="ExternalOutput")

    with TileContext(nc) as tc:
        with (
            tc.tile_pool(name="const", bufs=1) as const_pool,
            tc.tile_pool(name="psum", bufs=1, space="PSUM") as psum_pool,
            tc.tile_pool(name="prod", bufs=cfg["prod_bufs"]) as prod_pool,
            tc.tile_pool(name="half", bufs=cfg["prod_bufs"]) as half_pool,
            tc.tile_pool(name="junk", bufs=cfg["junk_bufs"]) as junk_pool,
        ):
            idx_sb = const_pool.tile([128, COLS], I16, name="idx_sb")
            nc.sync.dma_start(out=idx_sb[:], in_=idx_d[:])
            cg_sb = const_pool.tile([128, NU * NQB, C], BF16, name="cg_sb")
            nc.gpsimd.dma_gather(
                cg_sb[:], call_d[:], idx_sb[:], NI, NI, C, single_packet=False
            )

            zt_sb = const_pool.tile([128, 4, ZT_PAD], BF16, name="zt_sb")
            nc.sync.dma_start(out=zt_sb[:], in_=zt_d[:])
            wk_sb = const_pool.tile([128, 4, K, C], BF16, name="wk_sb")
            for h in range(2):
                eng = nc.sync if h == 0 else nc.scalar
                eng.dma_start(
                    out=wk_sb[:, :, h * 6 : (h + 1) * 6],
                    in_=wk_d[:, :, h * 6 : (h + 1) * 6],
                )

            # zps[q_loc, qj, k, :] for this core's two q-blocks
            zps = const_pool.tile([128, NQB, K, C], BF16, name="zps")
            for qj in range(NQB):
                for h in range(2):  # k-halves of 6
                    ps = psum_pool.tile(
                        [128, 6, C], F32, name=f"ps{qj}_{h}", tag=f"ps{(qj * 2 + h) % 2}"
                    )
                    for kk in range(6):
                        k = h * 6 + kk
                        # lhsT columns: q0 + k + [0,128); q0 baked per-core
                        # via the qj dimension of zt's layout? No: q0 differs
                        # per core only through the DATA (zt columns), so the
                        # program uses local offset qj*128 into a per-core
                        # pre-shifted zt. zt holds columns for q in
                        # [g*256, g*256+256+shift): host uploads the window.
                        for dc in range(4):
                            nc.tensor.matmul(
                                ps[:, kk],
                                zt_sb[:, dc, qj * 128 + k : qj * 128 + k + 128],
                                wk_sb[:, dc, k, :],
                                start=(dc == 0),
                                stop=(dc == 3),
                            )
                    nc.scalar.copy(zps[:, qj, h * 6 : (h + 1) * 6, :], ps[:])

            scores = const_pool.tile([128, NU, NQB, K], F32, name="scores")

            for u in range(NU):
                for qj in range(NQB):
                    i = u * NQB + qj
                    mode = modes[i]
                    cgi = cg_sb[:, i, :]
                    mul_eng = nc.gpsimd if mode in ("g", "h") else nc.vector
                    prod = prod_pool.tile([128, K, C], BF16, tag="pr", name=f"pr{i}")
                    mul_eng.tensor_tensor(
                        out=prod[:],
                        in0=cgi.unsqueeze(1).broadcast_to([128, K, C]),
                        in1=zps[:, qj],
                        op=mybir.AluOpType.mult,
                    )
                    h1 = half_pool.tile([128, K, C // 2], BF16, tag="h1", name=f"h1{i}")
                    nc.vector.tensor_tensor(
                        out=h1[:], in0=prod[:, :, : C // 2], in1=prod[:, :, C // 2 :],
                        op=mybir.AluOpType.add,
                    )
                    h2 = half_pool.tile([128, K, C // 4], BF16, tag="h2", name=f"h2{i}")
                    nc.vector.tensor_tensor(
                        out=h2[:], in0=h1[:, :, : C // 4], in1=h1[:, :, C // 4 :],
                        op=mybir.AluOpType.add,
                    )
                    if mode in ("d", "g"):
                        nc.vector.tensor_reduce(
                            out=scores[:, u, qj, :], in_=h2[:],
                            axis=mybir.AxisListType.X, op=mybir.AluOpType.add,
                        )
                    else:  # 'a' / 'h'
                        for k in range(K):
                            junk = junk_pool.tile(
                                [128, C // 4], BF16, tag="aj", name=f"aj{i}_{k}"
                            )
                            nc.scalar.activation(
                                out=junk[:], in_=h2[:, k, :],
                                func=mybir.ActivationFunctionType.Copy,
                                accum_out=scores[:, u, qj, k : k + 1],
                            )

            nc.sync.dma_start(out=out_d[:], in_=scores[:])

    nc.compile()
    if cfg == CFG:
        _NC = nc
    return nc


def _unit_perms(perms_len, perm_L, perm_B, b_src):
    """Per-unit (b_out, channel, forward-perm sl(l), inverse-perm l(sl))."""
    ident = np.arange(LW)
    inv_len = [np.argsort(perms_len[n]) for n in range(B)]
    inv_pl = np.argsort(perm_L)
    n_src = int(np.nonzero(perm_B == b_src)[0][0])
    units = [(b_src, 0, ident, ident)]
    for n in range(B):
        units.append((b_src, 1 + B + n, perms_len[n], inv_len[n]))
    for b_out in range(B):
        units.append((b_out, 1 + n_src, perm_L, inv_pl))
    return units


def _make_inputs(c, z, Wk, perms_len, perm_L, perm_B):
    """Host-side prep: transposed/padded operands + baked gather indices."""
    c_all = np.ascontiguousarray(c.reshape(B * L, C)).astype(BF16_NP)
    wk_dc = np.ascontiguousarray(
        Wk.reshape(K, 4, 128, C).transpose(2, 1, 0, 3)
    ).astype(BF16_NP)  # [128, 4, K, C]

    zt_full = np.zeros((B, 128, 4, ZT_PAD), dtype=BF16_NP)
    for b in range(B):
        # zt[dp, dc, r] = z[b, r, dc*128+dp]
        zt = z[b].T.reshape(4, 128, L).transpose(1, 0, 2)  # [128, 4, 512]
        zt_full[b, :, :, :L] = zt.astype(BF16_NP)

    in_maps = []
    for b_src in range(B):
        units = _unit_perms(perms_len, perm_L, perm_B, b_src)
        for g in range(2):
            # zt window: program reads columns qj*128 + k + [0,128) for
            # qj in {0,1}; global q0 = g*256, so upload columns
            # [g*256, g*256 + 256 + 16) left-aligned.
            ztw = np.zeros((128, 4, ZT_PAD), dtype=BF16_NP)
            lo = g * 256
            hi = min(L, lo + ZT_PAD)
            ztw[:, :, : hi - lo] = zt_full[b_src, :, :, lo:hi]

            idx = np.zeros((128, COLS), np.int16)
            s = np.arange(NQB * 128)
            for u, (b_out, _ch, _fwd, inv) in enumerate(units):
                q_glob = g * 256 + s
                sl = q_glob - 1
                valid = (sl >= 0) & (sl < LW)
                lvals = np.zeros(NQB * 128, np.int64)
                lvals[valid] = inv[sl[valid]]
                vals = np.where(valid, b_out * L + lvals, 0).astype(np.int16)
                # slot s_glob = u*256 + s -> idx[16*grp + s_glob%16, s_glob//16]
                col = u * 16 + s // 16
                row = s % 16
                for grp in range(8):
                    idx[16 * grp + row, col] = vals
            in_maps.append({"zt": ztw, "wk": wk_dc, "call": c_all, "idx": idx})
    return in_maps


def kernel(c, z, Wk, perms_len, perm_L, perm_B, _trace=False, _result_holder=None):
    c = np.asarray(c, np.float32)
    z = np.asarray(z, np.float32)
    Wk = np.asarray(Wk, np.float32)
    perms_len = np.asarray(perms_len, np.int64)
    perm_L = np.asarray(perm_L, np.int64)
    perm_B = np.asarray(perm_B, np.int64)

    nc = _build_program()
    in_maps = _make_inputs(c, z, Wk, perms_len, perm_L, perm_B)
    res = bass_utils.run_bass_kernel_spmd(
        nc, in_maps, core_ids=list(range(2 * B)), trace=_trace
    )
    if _result_holder is not None:
        _result_holder.append(res)

    out = np.empty((B, NM, LW, K), np.float32)
    larr = np.arange(LW)
    for b_src in range(B):
        units = _unit_perms(perms_len, perm_L, perm_B, b_src)
        for g in range(2):
            co = res.results[2 * b_src + g]["out"]  # [128, NU, NQB, K]
            for u, (b_out, ch, fwd, _inv) in enumerate(units):
                q = fwd + 1
                qb = q // 128
                sel = (qb // 2) == g
                out[b_out, ch, larr[sel]] = co[q[sel] % 128, u, qb[sel] % 2, :]
    return out


# revision 11
# speedup vs baseline: 2.0791x; 1.0462x over previous
"""Trainium2 Bass kernel for nn_PredictionModel (CPC-style prediction scores).

Reference computation (B=4, L=512, D=512, C=256, K=12, LW=500):
  cp[b,l,k,:]    = c[b,l,:] @ Wk[k].T            (row of R^D)
  zw[b,l,k,:]    = z[b, l+1+k, :]
  pos[b,l,k]     = <cp[b,l,k], zw[b,l,k]>
  neg_g[b,n,l,k] = <cp[b,l,k], zw[perm_B[n], perm_L[l], k]>
  neg_len[b,n,l,k]=<cp[b,l,k], zw[b, perms_len[n,l], k]>
  out = concat([pos[:,None], neg_g, neg_len], axis=1)   # (B, 9, LW, K)

Key algebraic move (C-space dots): <c[l] @ Wk[k].T, z[r]> = <c[l], z[r] @ Wk[k]>.
Define zp[r,k,:] = z[r,:] @ Wk[k] in R^C and the k-shifted table
zps[q,k,:] = zp[q+k,k,:]; then every score is
  score[...,l,k] = <c[b_out, l, :], zps_{b_src}[perm(l)+1, k, :]>
so one 6KB-contiguous row of zps serves all 12 k of a score row, and the
dot length is C=256 instead of D=512.

Per-core plan (8 cores = 4 source-batches x 2 q-halves):
  - PE computes zps_{b_src} for its 2 q-blocks directly in shifted layout
    (lhsT = z^T columns offset by k; 96 matmuls, bf16).
  - One dma_gather pulls the c rows (512B each) for all 9 units straight
    from HBM in q-natural order (indices baked host-side from the perms).
  - Dots: bf16 mul (c broadcast over k) + halving adds + reduce, split
    across DVE / ACT(accum) / Pool(fused stt) by a tunable mode string.
  - Scores are emitted q-indexed; the host un-permutes (pure indexing).
"""

import numpy as np
import ml_dtypes

import concourse.mybir as mybir
from concourse import bacc
from concourse.tile import TileContext
from concourse import bass_utils

B, L, D, C, K = 4, 512, 512, 256, 12
LW = L - K            # 500
NM = 2 * B + 1        # 9 output channels
NU = 9                # units per source batch
NQB = 2               # q-blocks per core
NI = NU * NQB * 128   # gather slots per core = 2304
COLS = NI // 16       # idx columns = 144
ZT_PAD = 528          # 512 + 16 (k-shift slack)
F32 = mybir.dt.float32
BF16 = mybir.dt.bfloat16
I16 = mybir.dt.int16
BF16_NP = ml_dtypes.bfloat16

_NC = None

# engine mode per (unit, qj) flat index i = u*2+qj:
#   'd' = DVE mul+halve+halve+reduce
#   'a' = DVE mul+halve+halve, ACT per-k accum tail
#   'p' = Pool fused scalar_tensor_tensor per k
CFG = {
    "modes": "dpamdpamdpamdpampa",
    "prod_bufs": 4,
    "junk_bufs": 4,
}


def _build_program(cfg=None):
    """One NeuronCore program, identical across the 8 cores."""
    global _NC
    if cfg is None and _NC is not None:
        return _NC
    cfg = {**CFG, **(cfg or {})}
    modes = cfg["modes"]
    assert len(modes) == NU * NQB

    nc = bacc.Bacc()
    # z[b_src]^T padded: [128 d-part, 4 d-chunk, 528 r]
    zt_d = nc.dram_tensor("zt", [128, 4, ZT_PAD], BF16, kind="ExternalInput")
    # Wk transposed: [128 d-part, 4 d-chunk, K, C]
    wk_d = nc.dram_tensor("wk", [128, 4, K, C], BF16, kind="ExternalInput")
    # all batches' c rows: [B*L, C]
    call_d = nc.dram_tensor("call", [B * L, C], BF16, kind="ExternalInput")
    # gather index table (wrap-16 slots, replicated to all 8 Q7 groups)
    idx_d = nc.dram_tensor("idx", [128, COLS], I16, kind="ExternalInput")
    out_d = nc.dram_tensor("out", [128, NU, NQB, K], F32, kind="ExternalOutput")

    with TileContext(nc) as tc:
        with (
            tc.tile_pool(name="const", bufs=1) as const_pool,
            tc.tile_pool(name="psum", bufs=1, space="PSUM") as psum_pool,
            tc.tile_pool(name="prod", bufs=cfg["prod_bufs"]) as prod_pool,
            tc.tile_pool(name="half", bufs=cfg["prod_bufs"]) as half_pool,
            tc.tile_pool(name="junk", bufs=cfg["junk_bufs"]) as junk_pool,
        ):
            idx_sb = const_pool.tile([128, COLS], I16, name="idx_sb")
            nc.sync.dma_start(out=idx_sb[:], in_=idx_d[:])
            cg_sb = const_pool.tile([128, NU * NQB, C], BF16, name="cg_sb")
            nc.gpsimd.dma_gather(
                cg_sb[:], call_d[:], idx_sb[:], NI, NI, C, single_packet=False
            )

            zt_sb = const_pool.tile([128, 4, ZT_PAD], BF16, name="zt_sb")
            nc.sync.dma_start(out=zt_sb[:], in_=zt_d[:])
            wk_sb = const_pool.tile([128, 4, K, C], BF16, name="wk_sb")
            for h in range(2):
                eng = nc.sync if h == 0 else nc.scalar
                eng.dma_start(
                    out=wk_sb[:, :, h * 6 : (h + 1) * 6],
                    in_=wk_d[:, :, h * 6 : (h + 1) * 6],
                )

            # zps for this core's two q-blocks, fold-friendly layout:
            # [128, qj, a, b, g, k, cc] with c' = a*128 + b*64 + g*32 + cc
            zps = const_pool.tile([128, NQB, 2, 2, 2, K, C // 8], BF16, name="zps")
            for qj in range(NQB):
                for h in range(2):  # k-halves of 6
                    ps = psum_pool.tile(
                        [128, 6, C], F32, name=f"ps{qj}_{h}", tag=f"ps{(qj * 2 + h) % 2}"
                    )
                    for kk in range(6):
                        k = h * 6 + kk
                        # lhsT columns: q0 + k + [0,128); q0 baked per-core
                        # via the qj dimension of zt's layout? No: q0 differs
                        # per core only through the DATA (zt columns), so the
                        # program uses local offset qj*128 into a per-core
                        # pre-shifted zt. zt holds columns for q in
                        # [g*256, g*256+256+shift): host uploads the window.
                        for dc in range(4):
                            nc.tensor.matmul(
                                ps[:, kk],
                                zt_sb[:, dc, qj * 128 + k : qj * 128 + k + 128],
                                wk_sb[:, dc, k, :],
                                start=(dc == 0),
                                stop=(dc == 3),
                            )
                    nc.scalar.copy(
                        zps[:, qj, :, :, :, h * 6 : (h + 1) * 6, :],
                        ps[:].rearrange(
                            "p k (a b g cc) -> p a b g k cc", a=2, b=2, g=2
                        ),
                    )

            scores = const_pool.tile([128, NU, NQB, K], F32, name="scores")

            for u in range(NU):
                for qj in range(NQB):
                    i = u * NQB + qj
                    mode = modes[i]
                    cgb = (
                        cg_sb[:, i, :]
                        .rearrange("p (a b g cc) -> p a b g cc", a=2, b=2, g=2)
                        .unsqueeze(4)
                        .broadcast_to([128, 2, 2, 2, K, C // 8])
                    )
                    prod = prod_pool.tile(
                        [128, 2, 2, 2, K, C // 8], BF16, tag="pr", name=f"pr{i}"
                    )
                    nc.vector.tensor_tensor(
                        out=prod[:], in0=cgb, in1=zps[:, qj],
                        op=mybir.AluOpType.mult,
                    )
                    if mode == "a":
                        # ACT accumulates each k directly from prod
                        for k in range(K):
                            junk = junk_pool.tile(
                                [128, 2, 2, 2, C // 8], BF16, tag="aj",
                                name=f"aj{i}_{k}"
                            )
                            nc.scalar.activation(
                                out=junk[:],
                                in_=prod[:, :, :, :, k, :],
                                func=mybir.ActivationFunctionType.Copy,
                                accum_out=scores[:, u, qj, k : k + 1],
                            )
                        continue
                    if mode == "m":
                        # DMA-engine in-place folds, DVE final reduce
                        nc.gpsimd.dma_start(
                            out=prod[:, 0], in_=prod[:, 1],
                            accum_op=mybir.AluOpType.add,
                        )
                        nc.gpsimd.dma_start(
                            out=prod[:, 0, 0], in_=prod[:, 0, 1],
                            accum_op=mybir.AluOpType.add,
                        )
                        nc.gpsimd.dma_start(
                            out=prod[:, 0, 0, 0], in_=prod[:, 0, 0, 1],
                            accum_op=mybir.AluOpType.add,
                        )
                        nc.vector.tensor_reduce(
                            out=scores[:, u, qj, :], in_=prod[:, 0, 0, 0],
                            axis=mybir.AxisListType.X, op=mybir.AluOpType.add,
                        )
                        continue
                    fold_eng = nc.gpsimd if mode == "p" else nc.vector
                    h1 = half_pool.tile(
                        [128, 2, 2, K, C // 8], BF16, tag="h1", name=f"h1{i}"
                    )
                    fold_eng.tensor_tensor(
                        out=h1[:], in0=prod[:, 0], in1=prod[:, 1],
                        op=mybir.AluOpType.add,
                    )
                    h2 = half_pool.tile(
                        [128, 2, K, C // 8], BF16, tag="h2", name=f"h2{i}"
                    )
                    fold_eng.tensor_tensor(
                        out=h2[:], in0=h1[:, 0], in1=h1[:, 1],
                        op=mybir.AluOpType.add,
                    )
                    h3 = half_pool.tile(
                        [128, K, C // 8], BF16, tag="h3", name=f"h3{i}"
                    )
                    fold_eng.tensor_tensor(
                        out=h3[:], in0=h2[:, 0], in1=h2[:, 1],
                        op=mybir.AluOpType.add,
                    )
                    nc.vector.tensor_reduce(
                        out=scores[:, u, qj, :], in_=h3[:],
                        axis=mybir.AxisListType.X, op=mybir.AluOpType.add,
                    )

            nc.sync.dma_start(out=out_d[:], in_=scores[:])

    nc.compile()
    if cfg == CFG:
        _NC = nc
    return nc


def _unit_perms(perms_len, perm_L, perm_B, b_src):
    """Per-unit (b_out, channel, forward-perm sl(l), inverse-perm l(sl))."""
    ident = np.arange(LW)
    inv_len = [np.argsort(perms_len[n]) for n in range(B)]
    inv_pl = np.argsort(perm_L)
    n_src = int(np.nonzero(perm_B == b_src)[0][0])
    units = [(b_src, 0, ident, ident)]
    for n in range(B):
        units.append((b_src, 1 + B + n, perms_len[n], inv_len[n]))
    for b_out in range(B):
        units.append((b_out, 1 + n_src, perm_L, inv_pl))
    return units


def _make_inputs(c, z, Wk, perms_len, perm_L, perm_B):
    """Host-side prep: transposed/padded operands + baked gather indices."""
    c_all = np.ascontiguousarray(c.reshape(B * L, C)).astype(BF16_NP)
    wk_dc = np.ascontiguousarray(
        Wk.reshape(K, 4, 128, C).transpose(2, 1, 0, 3)
    ).astype(BF16_NP)  # [128, 4, K, C]

    zt_full = np.zeros((B, 128, 4, ZT_PAD), dtype=BF16_NP)
    for b in range(B):
        # zt[dp, dc, r] = z[b, r, dc*128+dp]
        zt = z[b].T.reshape(4, 128, L).transpose(1, 0, 2)  # [128, 4, 512]
        zt_full[b, :, :, :L] = zt.astype(BF16_NP)

    in_maps = []
    for b_src in range(B):
        units = _unit_perms(perms_len, perm_L, perm_B, b_src)
        for g in range(2):
            # zt window: program reads columns qj*128 + k + [0,128) for
            # qj in {0,1}; global q0 = g*256, so upload columns
            # [g*256, g*256 + 256 + 16) left-aligned.
            ztw = np.zeros((128, 4, ZT_PAD), dtype=BF16_NP)
            lo = g * 256
            hi = min(L, lo + ZT_PAD)
            ztw[:, :, : hi - lo] = zt_full[b_src, :, :, lo:hi]

            idx = np.zeros((128, COLS), np.int16)
            s = np.arange(NQB * 128)
            for u, (b_out, _ch, _fwd, inv) in enumerate(units):
                q_glob = g * 256 + s
                sl = q_glob - 1
                valid = (sl >= 0) & (sl < LW)
                lvals = np.zeros(NQB * 128, np.int64)
                lvals[valid] = inv[sl[valid]]
                vals = np.where(valid, b_out * L + lvals, 0).astype(np.int16)
                # slot s_glob = u*256 + s -> idx[16*grp + s_glob%16, s_glob//16]
                col = u * 16 + s // 16
                row = s % 16
                for grp in range(8):
                    idx[16 * grp + row, col] = vals
            in_maps.append({"zt": ztw, "wk": wk_dc, "call": c_all, "idx": idx})
    return in_maps


def kernel(c, z, Wk, perms_len, perm_L, perm_B, _trace=False, _result_holder=None):
    c = np.asarray(c, np.float32)
    z = np.asarray(z, np.float32)
    Wk = np.asarray(Wk, np.float32)
    perms_len = np.asarray(perms_len, np.int64)
    perm_L = np.asarray(perm_L, np.int64)
    perm_B = np.asarray(perm_B, np.int64)

    nc = _build_program()
    in_maps = _make_inputs(c, z, Wk, perms_len, perm_L, perm_B)
    res = bass_utils.run_bass_kernel_spmd(
        nc, in_maps, core_ids=list(range(2 * B)), trace=_trace
    )
    if _result_holder is not None:
        _result_holder.append(res)

    out = np.empty((B, NM, LW, K), np.float32)
    larr = np.arange(LW)
    for b_src in range(B):
        units = _unit_perms(perms_len, perm_L, perm_B, b_src)
        for g in range(2):
            co = res.results[2 * b_src + g]["out"]  # [128, NU, NQB, K]
            for u, (b_out, ch, fwd, _inv) in enumerate(units):
                q = fwd + 1
                qb = q // 128
                sel = (qb // 2) == g
                out[b_out, ch, larr[sel]] = co[q[sel] % 128, u, qb[sel] % 2, :]
    return out


# revision 14
# speedup vs baseline: 2.1315x; 1.0252x over previous
"""Trainium2 Bass kernel for nn_PredictionModel (CPC-style prediction scores).

Reference computation (B=4, L=512, D=512, C=256, K=12, LW=500):
  cp[b,l,k,:]    = c[b,l,:] @ Wk[k].T            (row of R^D)
  zw[b,l,k,:]    = z[b, l+1+k, :]
  pos[b,l,k]     = <cp[b,l,k], zw[b,l,k]>
  neg_g[b,n,l,k] = <cp[b,l,k], zw[perm_B[n], perm_L[l], k]>
  neg_len[b,n,l,k]=<cp[b,l,k], zw[b, perms_len[n,l], k]>
  out = concat([pos[:,None], neg_g, neg_len], axis=1)   # (B, 9, LW, K)

Key algebraic move (C-space dots): <c[l] @ Wk[k].T, z[r]> = <c[l], z[r] @ Wk[k]>.
Define zp[r,k,:] = z[r,:] @ Wk[k] in R^C and the k-shifted table
zps[q,k,:] = zp[q+k,k,:]; then every score is
  score[...,l,k] = <c[b_out, l, :], zps_{b_src}[perm(l)+1, k, :]>
so one 6KB-contiguous row of zps serves all 12 k of a score row, and the
dot length is C=256 instead of D=512.

Per-core plan (8 cores = 4 source-batches x 2 q-halves):
  - PE computes zps_{b_src} for its 2 q-blocks directly in shifted layout
    (lhsT = z^T columns offset by k; 96 matmuls, bf16).
  - One dma_gather pulls the c rows (512B each) for all 9 units straight
    from HBM in q-natural order (indices baked host-side from the perms).
  - Dots: bf16 mul (c broadcast over k) + halving adds + reduce, split
    across DVE / ACT(accum) / Pool(fused stt) by a tunable mode string.
  - Scores are emitted q-indexed; the host un-permutes (pure indexing).
"""

import numpy as np
import ml_dtypes

import concourse.mybir as mybir
from concourse import bacc
from concourse.tile import TileContext
from concourse import bass_utils

B, L, D, C, K = 4, 512, 512, 256, 12
LW = L - K            # 500
NM = 2 * B + 1        # 9 output channels
NU = 9                # units per source batch
NQB = 2               # q-blocks per core
NI = NU * NQB * 128   # gather slots per core = 2304
COLS = NI // 16       # idx columns = 144
ZT_PAD = 528          # 512 + 16 (k-shift slack)
F32 = mybir.dt.float32
BF16 = mybir.dt.bfloat16
I16 = mybir.dt.int16
BF16_NP = ml_dtypes.bfloat16

_NC = None

# engine mode per (unit, qj) flat index i = u*2+qj:
#   'd' = DVE mul+halve+halve+reduce
#   'a' = DVE mul+halve+halve, ACT per-k accum tail
#   'p' = Pool fused scalar_tensor_tensor per k
CFG = {
    "modes": "dpadpadpadpadpaPdP",
    "prod_bufs": 6,
    "junk_bufs": 4,
}


def _build_program(cfg=None):
    """One NeuronCore program, identical across the 8 cores."""
    global _NC
    if cfg is None and _NC is not None:
        return _NC
    cfg = {**CFG, **(cfg or {})}
    modes = cfg["modes"]
    assert len(modes) == NU * NQB

    nc = bacc.Bacc()
    # z[b_src]^T padded: [128 d-part, 4 d-chunk, 528 r]
    zt_d = nc.dram_tensor("zt", [128, 4, ZT_PAD], BF16, kind="ExternalInput")
    # Wk transposed: [128 d-part, 4 d-chunk, K, C]
    wk_d = nc.dram_tensor("wk", [128, 4, K, C], BF16, kind="ExternalInput")
    # all batches' c rows: [B*L, C]
    call_d = nc.dram_tensor("call", [B * L, C], BF16, kind="ExternalInput")
    # gather index table (wrap-16 slots, replicated to all 8 Q7 groups)
    idx_d = nc.dram_tensor("idx", [128, COLS], I16, kind="ExternalInput")
    out_d = nc.dram_tensor("out", [128, NU, NQB, K], F32, kind="ExternalOutput")

    with TileContext(nc) as tc:
        with (
            tc.tile_pool(name="const", bufs=1) as const_pool,
            tc.tile_pool(name="psum", bufs=1, space="PSUM") as psum_pool,
            tc.tile_pool(name="prod", bufs=cfg["prod_bufs"]) as prod_pool,
            tc.tile_pool(name="half", bufs=cfg["prod_bufs"]) as half_pool,
            tc.tile_pool(name="junk", bufs=cfg["junk_bufs"]) as junk_pool,
        ):
            zt_sb = const_pool.tile([128, 4, ZT_PAD], BF16, name="zt_sb")
            nc.sync.dma_start(out=zt_sb[:], in_=zt_d[:])
            wk_sb = const_pool.tile([128, 4, K, C], BF16, name="wk_sb")
            nc.scalar.dma_start(out=wk_sb[:, 0], in_=wk_d[:, 0])
            idx_sb = const_pool.tile([128, COLS], I16, name="idx_sb")
            nc.sync.dma_start(out=idx_sb[:], in_=idx_d[:])
            cg_sb = const_pool.tile([128, NU * NQB, C], BF16, name="cg_sb")
            nc.gpsimd.dma_gather(
                cg_sb[:], call_d[:], idx_sb[:], NI, NI, C, single_packet=False
            )
            for dc in range(1, 4):
                eng = nc.sync if dc % 2 == 0 else nc.scalar
                eng.dma_start(out=wk_sb[:, dc], in_=wk_d[:, dc])

            # zps for this core's two q-blocks, fold-friendly layout:
            # [128, qj, a, b, g, k, cc] with c' = a*128 + b*64 + g*32 + cc
            zps = [
                const_pool.tile([128, 2, 2, 2, K, C // 8], BF16, name=f"zps{qj}")
                for qj in range(NQB)
            ]
            for qj in range(NQB):
                for h in range(2):  # k-halves of 6
                    ps = psum_pool.tile(
                        [128, 6, C], F32, name=f"ps{qj}_{h}", tag=f"ps{(qj * 2 + h) % 2}"
                    )
                    for kk in range(6):
                        k = h * 6 + kk
                        for dc in range(4):
                            nc.tensor.matmul(
                                ps[:, kk],
                                zt_sb[:, dc, qj * 128 + k : qj * 128 + k + 128],
                                wk_sb[:, dc, k, :],
                                start=(dc == 0),
                                stop=(dc == 3),
                            )
                    nc.scalar.copy(
                        zps[qj][:, :, :, :, h * 6 : (h + 1) * 6, :],
                        ps[:].rearrange(
                            "p k (a b g cc) -> p a b g k cc", a=2, b=2, g=2
                        ),
                    )

            scores = {
                (u, qj): const_pool.tile([128, K], F32, name=f"sc{u}_{qj}")
                for u in range(NU)
                for qj in range(NQB)
            }

            for u in range(NU):
                for qj in range(NQB):
                    i = u * NQB + qj
                    mode = modes[i]
                    cgb = (
                        cg_sb[:, i, :]
                        .rearrange("p (a b g cc) -> p a b g cc", a=2, b=2, g=2)
                        .unsqueeze(4)
                        .broadcast_to([128, 2, 2, 2, K, C // 8])
                    )
                    prod = prod_pool.tile(
                        [128, 2, 2, 2, K, C // 8], BF16, tag="pr", name=f"pr{i}"
                    )
                    mul_eng = nc.gpsimd if mode == "P" else nc.vector
                    mul_eng.tensor_tensor(
                        out=prod[:], in0=cgb, in1=zps[qj][:],
                        op=mybir.AluOpType.mult,
                    )
                    if mode == "a":
                        # ACT accumulates each k directly from prod
                        for k in range(K):
                            junk = junk_pool.tile(
                                [128, 2, 2, 2, C // 8], BF16, tag="aj",
                                name=f"aj{i}_{k}"
                            )
                            nc.scalar.activation(
                                out=junk[:],
                                in_=prod[:, :, :, :, k, :],
                                func=mybir.ActivationFunctionType.Copy,
                                accum_out=scores[(u, qj)][:, k : k + 1],
                            )
                        continue
                    fold_eng = nc.gpsimd if mode == "p" else nc.vector
                    h1 = half_pool.tile(
                        [128, 2, 2, K, C // 8], BF16, tag="h1", name=f"h1{i}"
                    )
                    fold_eng.tensor_tensor(
                        out=h1[:], in0=prod[:, 0], in1=prod[:, 1],
                        op=mybir.AluOpType.add,
                    )
                    h2 = half_pool.tile(
                        [128, 2, K, C // 8], BF16, tag="h2", name=f"h2{i}"
                    )
                    fold_eng.tensor_tensor(
                        out=h2[:], in0=h1[:, 0], in1=h1[:, 1],
                        op=mybir.AluOpType.add,
                    )
                    h3 = half_pool.tile(
                        [128, K, C // 8], BF16, tag="h3", name=f"h3{i}"
                    )
                    fold_eng.tensor_tensor(
                        out=h3[:], in0=h2[:, 0], in1=h2[:, 1],
                        op=mybir.AluOpType.add,
                    )
                    nc.vector.tensor_reduce(
                        out=scores[(u, qj)][:], in_=h3[:],
                        axis=mybir.AxisListType.X, op=mybir.AluOpType.add,
                    )

            for u in range(NU):
                for qj in range(NQB):
                    eng = nc.sync if (u * NQB + qj) % 2 == 0 else nc.scalar
                    eng.dma_start(
                        out=out_d[:, u, qj], in_=scores[(u, qj)][:]
                    )

    nc.compile()
    if cfg == CFG:
        _NC = nc
    return nc


def _unit_perms(perms_len, perm_L, perm_B, b_src):
    """Per-unit (b_out, channel, forward-perm sl(l), inverse-perm l(sl))."""
    ident = np.arange(LW)
    inv_len = [np.argsort(perms_len[n]) for n in range(B)]
    inv_pl = np.argsort(perm_L)
    n_src = int(np.nonzero(perm_B == b_src)[0][0])
    units = [(b_src, 0, ident, ident)]
    for n in range(B):
        units.append((b_src, 1 + B + n, perms_len[n], inv_len[n]))
    for b_out in range(B):
        units.append((b_out, 1 + n_src, perm_L, inv_pl))
    return units


def _make_inputs(c, z, Wk, perms_len, perm_L, perm_B):
    """Host-side prep: transposed/padded operands + baked gather indices."""
    c_all = np.ascontiguousarray(c.reshape(B * L, C)).astype(BF16_NP)
    wk_dc = np.ascontiguousarray(
        Wk.reshape(K, 4, 128, C).transpose(2, 1, 0, 3)
    ).astype(BF16_NP)  # [128, 4, K, C]

    zt_full = np.zeros((B, 128, 4, ZT_PAD), dtype=BF16_NP)
    for b in range(B):
        # zt[dp, dc, r] = z[b, r, dc*128+dp]
        zt = z[b].T.reshape(4, 128, L).transpose(1, 0, 2)  # [128, 4, 512]
        zt_full[b, :, :, :L] = zt.astype(BF16_NP)

    in_maps = []
    for b_src in range(B):
        units = _unit_perms(perms_len, perm_L, perm_B, b_src)
        for g in range(2):
            # zt window: program reads columns qj*128 + k + [0,128) for
            # qj in {0,1}; global q0 = g*256, so upload columns
            # [g*256, g*256 + 256 + 16) left-aligned.
            ztw = np.zeros((128, 4, ZT_PAD), dtype=BF16_NP)
            lo = g * 256
            hi = min(L, lo + ZT_PAD)
            ztw[:, :, : hi - lo] = zt_full[b_src, :, :, lo:hi]

            idx = np.zeros((128, COLS), np.int16)
            s = np.arange(NQB * 128)
            for u, (b_out, _ch, _fwd, inv) in enumerate(units):
                q_glob = g * 256 + s
                sl = q_glob - 1
                valid = (sl >= 0) & (sl < LW)
                lvals = np.zeros(NQB * 128, np.int64)
                lvals[valid] = inv[sl[valid]]
                vals = np.where(valid, b_out * L + lvals, 0).astype(np.int16)
                # slot s_glob = u*256 + s -> idx[16*grp + s_glob%16, s_glob//16]
                col = u * 16 + s // 16
                row = s % 16
                for grp in range(8):
                    idx[16 * grp + row, col] = vals
            in_maps.append({"zt": ztw, "wk": wk_dc, "call": c_all, "idx": idx})
    return in_maps


def kernel(c, z, Wk, perms_len, perm_L, perm_B, _trace=False, _result_holder=None):
    c = np.asarray(c, np.float32)
    z = np.asarray(z, np.float32)
    Wk = np.asarray(Wk, np.float32)
    perms_len = np.asarray(perms_len, np.int64)
    perm_L = np.asarray(perm_L, np.int64)
    perm_B = np.asarray(perm_B, np.int64)

    nc = _build_program()
    in_maps = _make_inputs(c, z, Wk, perms_len, perm_L, perm_B)
    res = bass_utils.run_bass_kernel_spmd(
        nc, in_maps, core_ids=list(range(2 * B)), trace=_trace
    )
    if _result_holder is not None:
        _result_holder.append(res)

    out = np.empty((B, NM, LW, K), np.float32)
    larr = np.arange(LW)
    for b_src in range(B):
        units = _unit_perms(perms_len, perm_L, perm_B, b_src)
        for g in range(2):
            co = res.results[2 * b_src + g]["out"]  # [128, NU, NQB, K]
            for u, (b_out, ch, fwd, _inv) in enumerate(units):
                q = fwd + 1
                qb = q // 128
                sel = (qb // 2) == g
                out[b_out, ch, larr[sel]] = co[q[sel] % 128, u, qb[sel] % 2, :]
    return out
